# revision 9
# baseline (speedup 1.0000x reference)
"""GCN2 (GCNII) forward pass on 8 Trainium2 NeuronCores via Bass/Tile.

Strategy (node sharding per sharding hint):
  - dst nodes sharded across 8 cores; per-core slice padded to NLOC rows.
  - full h replica [NC*NLOC, H] bf16 in DRAM on every core, rebuilt each layer
    via AllGather (the "halo exchange" degenerate case: full replication).
  - aggregation: edges grouped by (dst-group of 512, src-core bucket of NLOC
    rows). Per (group, bucket) ONE dma_gather (int16 window indices) pulls
    MAIN_COLS*4+OVF_COLS columns of 128 messages; selection matrices S
    (precomputed on host, streamed from DRAM) turn segment-sum into PE
    matmuls accumulating aggT[f, 512] in PSUM.
  - epilogue folds the initial-residual term (x0a resident in SBUF), applies
    Wt = (1-beta)I + beta*W via one matmul per 128-node block, relu, writes
    the new h slice; AllGather rebuilds the replica.
  - last layer computes logits + log_softmax on-chip, f32 output.
"""
import math
import numpy as np

import concourse.bass as bass
import concourse.bacc as bacc
import concourse.tile as tile
import concourse.mybir as mybir
import concourse.bass_utils as bass_utils
from concourse.masks import make_identity

# ---------------- configuration ----------------
class CFG:
    NC = 8
    N = 100000
    F_IN = 512
    H = 128
    C = 40
    L = 8
    ALPHA = 0.1
    THETA = 0.5
    MAIN_COLS = 2      # 128-edge columns per (block, bucket)
    OVF_COLS = 1       # group-wide overflow columns per (group, bucket)
    GRP = 512
    BLK = 128

    @classmethod
    def derived(cls):
        cls.NPC = cls.N // cls.NC
        cls.NGRP = math.ceil(cls.NPC / cls.GRP)
        cls.NLOC = cls.NGRP * cls.GRP
        cls.NBLK = cls.GRP // cls.BLK
        cls.NCOLS_M = cls.NBLK * cls.MAIN_COLS
        cls.COLS = cls.NCOLS_M + cls.OVF_COLS
        cls.IDX_GJ = cls.COLS * 128
        cls.KT = cls.F_IN // 128


CFG.derived()


def set_sizes(N, F_IN=512, L=8, main_cols=2, ovf_cols=1):
    CFG.N = N
    CFG.F_IN = F_IN
    CFG.L = L
    CFG.MAIN_COLS = main_cols
    CFG.OVF_COLS = ovf_cols
    CFG.derived()


# ---------------- host preprocessing ----------------
def preprocess(edge_index, edge_weight):
    """Per-core device-feed arrays: wrapped int16 gather indices and dense
    selection matrices (bf16) with (1-alpha)-scaled weights baked in."""
    c_ = CFG
    src = edge_index[0].astype(np.int64)
    dst = edge_index[1].astype(np.int64)
    w = edge_weight.astype(np.float32) * (1.0 - c_.ALPHA)
    cores = []
    for c in range(c_.NC):
        m = (dst // c_.NPC) == c
        s_c, d_c, w_c = src[m], dst[m], w[m]
        dl = d_c - c * c_.NPC
        g = dl // c_.GRP
        b = (dl % c_.GRP) // c_.BLK
        j = s_c // c_.NPC
        ilocal = (s_c % c_.NPC).astype(np.int64)
        slot128 = dl % c_.BLK
        slot512 = dl % c_.GRP
        order = np.lexsort((ilocal, b, j, g))
        g, b, j, ilocal, slot128, slot512, w_c = (a[order] for a in (g, b, j, ilocal, slot128, slot512, w_c))
        key = (g * c_.NC + j)
        idxT = np.zeros((c_.NGRP * c_.NC, c_.IDX_GJ), np.int16)
        Smain = np.zeros((c_.NGRP * c_.NC, 128, c_.NCOLS_M * 128), np.float32)
        Sovf = np.zeros((c_.NGRP * c_.NC, 128, c_.OVF_COLS * c_.GRP), np.float32)
        max_ovf = 0
        # boundaries of (g,j) segments
        seg_start = np.searchsorted(key, np.arange(c_.NGRP * c_.NC), side="left")
        seg_end = np.searchsorted(key, np.arange(c_.NGRP * c_.NC), side="right")
        for gj in range(c_.NGRP * c_.NC):
            s0, s1 = seg_start[gj], seg_end[gj]
            if s0 == s1:
                continue
            bv, iv = b[s0:s1], ilocal[s0:s1]
            s1v, s5v, wv = slot128[s0:s1], slot512[s0:s1], w_c[s0:s1]
            ovf_sel = np.zeros(s1 - s0, bool)
            for bi in range(c_.NBLK):
                bm_idx = np.nonzero(bv == bi)[0]
                cap = c_.MAIN_COLS * 128
                take = bm_idx[:cap]
                base = bi * cap
                nt = len(take)
                idxT[gj, base:base + nt] = iv[take]
                ee = (base + np.arange(nt))
                Smain[gj, ee % 128, (ee // 128) * 128 + s1v[take]] = wv[take]
                if len(bm_idx) > cap:
                    ovf_sel[bm_idx[cap:]] = True
            ov = np.nonzero(ovf_sel)[0]
            ov = ov[np.argsort(iv[ov], kind="stable")]
            no = len(ov)
            max_ovf = max(max_ovf, no)
            if no > c_.OVF_COLS * 128:
                raise OverflowError(f"core {c} gj {gj}: ovf {no} > {c_.OVF_COLS*128}")
            ob = c_.NCOLS_M * 128
            idxT[gj, ob:ob + no] = iv[ov]
            ee = np.arange(no)
            Sovf[gj, ee % 128, (ee // 128) * c_.GRP + s5v[ov]] = wv[ov]
        # wrap idx: flat i -> [i % 16, i // 16], replicated across 8 groups of 16
        wrapped = idxT.reshape(c_.NGRP * c_.NC, c_.IDX_GJ // 16, 16).transpose(0, 2, 1)
        wr = wrapped.transpose(1, 0, 2).reshape(16, -1)
        idx_dev = np.zeros((128, c_.NGRP * c_.NC * (c_.IDX_GJ // 16)), np.int16)
        for q in range(8):
            idx_dev[16 * q:16 * q + 16, :] = wr
        cores.append(dict(idx=idx_dev, Smain=Smain, Sovf=Sovf, max_ovf=max_ovf))
    return cores


def _bf16(a):
    import ml_dtypes
    return np.asarray(a, dtype=ml_dtypes.bfloat16)


# ---------------- device program ----------------
def build_program(debug=False):
    c_ = CFG
    nc = bacc.Bacc("TRN2", target_bir_lowering=False, debug=False,
                   enable_asserts=True, num_devices=c_.NC, num_swdge_queues=4)
    bf = mybir.dt.bfloat16
    f32 = mybir.dt.float32
    x_t = nc.dram_tensor("x", [c_.NLOC, c_.F_IN], bf, kind="ExternalInput").ap()
    idx_t = nc.dram_tensor("idx", [128, c_.NGRP * c_.NC * (c_.IDX_GJ // 16)], mybir.dt.int16, kind="ExternalInput").ap()
    sm_t = nc.dram_tensor("Smain", [c_.NGRP * c_.NC, 128, c_.NCOLS_M * 128], bf, kind="ExternalInput").ap()
    so_t = nc.dram_tensor("Sovf", [c_.NGRP * c_.NC, 128, c_.OVF_COLS * c_.GRP], bf, kind="ExternalInput").ap()
    w0_t = nc.dram_tensor("W0", [c_.F_IN, c_.H], bf, kind="ExternalInput").ap()
    wt_t = nc.dram_tensor("Wt", [c_.L, c_.H, c_.H], bf, kind="ExternalInput").ap()
    w1_t = nc.dram_tensor("W1", [c_.H, c_.C], bf, kind="ExternalInput").ap()
    out_t = nc.dram_tensor("out", [c_.NLOC, c_.C], f32, kind="ExternalOutput").ap()
    if debug:
        dbg_h0 = nc.dram_tensor("dbg_h0", [c_.NLOC, c_.H], bf, kind="ExternalOutput").ap()
        dbg_rep = nc.dram_tensor("dbg_rep", [c_.NC * c_.NLOC, c_.H], bf, kind="ExternalOutput").ap()
        dbg_outT = nc.dram_tensor("dbg_outT", [128, c_.GRP], f32, kind="ExternalOutput").ap()
        dbg_gt = nc.dram_tensor("dbg_gt", [128, c_.COLS * 128], bf, kind="ExternalOutput").ap()

    slice_b = nc.dram_tensor("slice_b", [c_.NLOC, c_.H], bf, kind="Internal").ap()
    replica = nc.dram_tensor("replica", [c_.NC * c_.NLOC, c_.H], bf, kind="Internal", addr_space="Shared").ap()

    rg = [list(range(c_.NC))]

    with tile.TileContext(nc) as tc:
        with tc.tile_pool(name="res", bufs=1) as res, \
             tc.tile_pool(name="work", bufs=1) as work, \
             tc.tile_pool(name="psum", bufs=1, space="PSUM") as psum:
            # resident tensors
            idx = res.tile([128, c_.NGRP * c_.NC * (c_.IDX_GJ // 16)], mybir.dt.int16)
            nc.sync.dma_start(idx[:], idx_t[:])
            w0 = res.tile([128, c_.KT, c_.H], bf)
            nc.sync.dma_start(w0[:], w0_t[:].rearrange("(k p) f -> p k f", p=128))
            wt = res.tile([128, c_.L, c_.H], bf)
            nc.sync.dma_start(wt[:], wt_t[:].rearrange("l f j -> f l j"))
            w1 = res.tile([128, c_.C], bf)
            nc.sync.dma_start(w1[:], w1_t[:])
            ident = res.tile([128, 128], bf)
            make_identity(nc, ident[:])
            x0a = res.tile([128, c_.NLOC], bf)

            # ---------- layer 0: h0 = relu(x @ W0); x0a = alpha*h0 ----------
            ctx0 = nc.named_scope("layer0"); ctx0.__enter__()
            for g in range(c_.NGRP):
                ph0 = psum.tile([128, c_.GRP], f32, name=f"pga{g%2}", tag="pga", bufs=2, space="PSUM")
                for k in range(c_.KT):
                    xt = work.tile([128, c_.GRP], bf, name=f"xt{k%3}", tag="xt", bufs=3)
                    nc.sync.dma_start(out=xt[:], in_=x_t[g * c_.GRP:(g + 1) * c_.GRP, k * 128:(k + 1) * 128], transpose=True)
                    nc.tensor.matmul(out=ph0[:], lhsT=w0[:, k, :], rhs=xt[:],
                                     start=(k == 0), stop=(k == c_.KT - 1))
                nc.scalar.activation(out=x0a[:, g * c_.GRP:(g + 1) * c_.GRP], in_=ph0[:],
                                     func=mybir.ActivationFunctionType.Relu, scale=c_.ALPHA)
                h0T = work.tile([128, c_.GRP], bf, name=f"h0T{g%2}", tag="h0T", bufs=2)
                nc.scalar.activation(out=h0T[:], in_=ph0[:], func=mybir.ActivationFunctionType.Relu)
                for b in range(c_.NBLK):
                    ptr = psum.tile([128, 128], bf, name=f"ptr{b%2}", tag="p2", bufs=4, space="PSUM")
                    nc.tensor.transpose(out=ptr[:], in_=h0T[:, b * 128:(b + 1) * 128], identity=ident[:])
                    hrow = work.tile([128, 128], bf, name=f"hrow{b%2}", tag="hrow", bufs=4)
                    nc.vector.tensor_copy(out=hrow[:], in_=ptr[:])
                    nc.sync.dma_start(out=slice_b[g * c_.GRP + b * 128: g * c_.GRP + (b + 1) * 128, :], in_=hrow[:])
            ctx0.__exit__(None, None, None)
            ctxag = nc.named_scope("ag0"); ctxag.__enter__()
            nc.gpsimd.collective_compute(
                "AllGather", mybir.AluOpType.bypass, replica_groups=rg,
                ins=[slice_b[:]], outs=[replica[:]])
            ctxag.__exit__(None, None, None)
            if debug:
                dcp = work.tile([128, c_.H], bf, name="dcp", tag="hrow", bufs=4)
                for r in range(c_.NLOC // 128):
                    nc.sync.dma_start(out=dcp[:], in_=slice_b[r * 128:(r + 1) * 128, :])
                    nc.scalar.dma_start(out=dbg_h0[r * 128:(r + 1) * 128, :], in_=dcp[:])
                for r in range(c_.NC * c_.NLOC // 128):
                    nc.sync.dma_start(out=dcp[:], in_=replica[r * 128:(r + 1) * 128, :])
                    nc.scalar.dma_start(out=dbg_rep[r * 128:(r + 1) * 128, :], in_=dcp[:])

            # ---------- conv layers ----------
            for l in range(c_.L):
                last = (l == c_.L - 1)
                ctxl = nc.named_scope(f"conv{l}"); ctxl.__enter__()
                for g in range(c_.NGRP):
                    pga = psum.tile([128, c_.GRP], f32, name=f"pga{g%2}", tag="pga", bufs=2, space="PSUM")
                    pgb = psum.tile([128, c_.GRP], f32, name=f"pgb{g%2}", tag="pgb", bufs=2, space="PSUM")
                    nc.tensor.matmul(out=pga[:], lhsT=ident[:],
                                     rhs=x0a[:, g * c_.GRP:(g + 1) * c_.GRP],
                                     start=True, stop=False, skip_group_check=True)
                    for j in range(c_.NC):
                        gj = g * c_.NC + j
                        pagg = pga if j < 4 else pgb
                        gt = work.tile([128, c_.COLS, 128], bf, name=f"gt{j%4}", tag="gt", bufs=8)
                        nc.gpsimd.dma_gather(
                            out_ap=gt[:], in_ap=replica[j * c_.NLOC:(j + 1) * c_.NLOC, :],
                            idxs_ap=idx[:, gj * (c_.IDX_GJ // 16):(gj + 1) * (c_.IDX_GJ // 16)],
                            num_idxs=c_.IDX_GJ, num_idxs_reg=c_.IDX_GJ, elem_size=c_.H,
                            queue_num=j % 4, single_packet=False)
                        sm = work.tile([128, c_.NCOLS_M * 128], bf, name=f"sm{j%3}", tag="sm", bufs=3)
                        nc.sync.dma_start(out=sm[:], in_=sm_t[gj, :, :])
                        so = work.tile([128, c_.OVF_COLS * c_.GRP], bf, name=f"so{j%3}", tag="so", bufs=3)
                        nc.scalar.dma_start(out=so[:], in_=so_t[gj, :, :])
                        # j==4: overflow column first; its start=True (512-wide,
                        # covers the whole bank) resets bank B - no init matmul.
                        if j == 4:
                            for oc in range(c_.OVF_COLS):
                                nc.tensor.matmul(out=pagg[:], lhsT=gt[:, c_.NCOLS_M + oc, :],
                                                 rhs=so[:, oc * c_.GRP:(oc + 1) * c_.GRP],
                                                 start=(oc == 0), stop=False,
                                                 skip_group_check=True)
                        for cc in range(c_.NCOLS_M):
                            bi = cc // c_.MAIN_COLS
                            nc.tensor.matmul(out=pagg[:, bi * 128:(bi + 1) * 128],
                                             lhsT=gt[:, cc, :], rhs=sm[:, cc * 128:(cc + 1) * 128],
                                             start=False, stop=False, skip_group_check=True)
                        if j != 4:
                            for oc in range(c_.OVF_COLS):
                                nc.tensor.matmul(out=pagg[:], lhsT=gt[:, c_.NCOLS_M + oc, :],
                                                 rhs=so[:, oc * c_.GRP:(oc + 1) * c_.GRP],
                                                 start=False,
                                                 stop=((j == 3 or j == c_.NC - 1) and oc == c_.OVF_COLS - 1),
                                                 skip_group_check=True)
                    aggb = work.tile([128, c_.GRP], bf, name=f"aggb{g%2}", tag="aggb", bufs=2)
                    nc.scalar.activation(out=aggb[:], in_=pgb[:], func=mybir.ActivationFunctionType.Copy)
                    outT = work.tile([128, c_.GRP], bf, name=f"outT{g%2}", tag="outT", bufs=2)
                    nc.vector.tensor_tensor(out=outT[:], in0=pga[:], in1=aggb[:],
                                            op=mybir.AluOpType.add)
                    for b in range(c_.NBLK):
                        if not last:
                            p2 = psum.tile([128, 128], f32, name=f"p2{b%3}", tag="p2", bufs=4, space="PSUM")
                            nc.tensor.matmul(out=p2[:], lhsT=outT[:, b * 128:(b + 1) * 128],
                                             rhs=wt[:, l, :], start=True, stop=True)
                            hnew = work.tile([128, 128], bf, name=f"hnew{b%2}", tag="hrow", bufs=4)
                            nc.scalar.activation(out=hnew[:], in_=p2[:], func=mybir.ActivationFunctionType.Relu)
                            eng = nc.sync if b % 2 == 0 else nc.scalar
                            eng.dma_start(out=slice_b[g * c_.GRP + b * 128: g * c_.GRP + (b + 1) * 128, :], in_=hnew[:])
                        else:
                            p2 = psum.tile([128, 128], f32, name=f"p2{b%3}", tag="p2", bufs=4, space="PSUM")
                            nc.tensor.matmul(out=p2[:], lhsT=wt[:, l, :],
                                             rhs=outT[:, b * 128:(b + 1) * 128], start=True, stop=True)
                            h8T = work.tile([128, 128], bf, name=f"h8T{b%2}", tag="hrow", bufs=4)
                            nc.scalar.activation(out=h8T[:], in_=p2[:], func=mybir.ActivationFunctionType.Relu)
                            plg = psum.tile([128, 128], f32, name=f"plg{b%3}", tag="p2", bufs=4, space="PSUM")
                            nc.tensor.matmul(out=plg[:, :c_.C], lhsT=h8T[:], rhs=w1[:], start=True, stop=True)
                            negm = work.tile([128, 1], f32, name=f"negm{b%2}", tag="negm", bufs=4)
                            nc.vector.reduce_max(out=negm[:], in_=plg[:, :c_.C], axis=mybir.AxisListType.X, negate=True)
                            esc = work.tile([128, c_.C], bf, name=f"esc{b%2}", tag="esc", bufs=2)
                            ssum = work.tile([128, 1], f32, name=f"ssum{b%2}", tag="ssum", bufs=4)
                            nc.scalar.activation(out=esc[:], in_=plg[:, :c_.C], func=mybir.ActivationFunctionType.Exp,
                                                 bias=negm[:, :1], accum_out=ssum[:, :1])
                            lsum = work.tile([128, 1], f32, name=f"lsum{b%2}", tag="lsum", bufs=4)
                            nc.scalar.activation(out=lsum[:], in_=ssum[:], func=mybir.ActivationFunctionType.Ln)
                            fin = work.tile([128, c_.C], f32, name=f"fin{b%2}", tag="fin", bufs=4)
                            nc.vector.tensor_scalar(out=fin[:], in0=plg[:, :c_.C],
                                                    scalar1=negm[:, :1], scalar2=lsum[:, :1],
                                                    op0=mybir.AluOpType.add, op1=mybir.AluOpType.subtract)
                            eng = nc.sync if b % 2 == 0 else nc.scalar
                            eng.dma_start(out=out_t[g * c_.GRP + b * 128: g * c_.GRP + (b + 1) * 128, :], in_=fin[:])
                ctxl.__exit__(None, None, None)
                if not last:
                    ctxa = nc.named_scope(f"ag{l+1}"); ctxa.__enter__()
                    nc.gpsimd.collective_compute(
                        "AllGather", mybir.AluOpType.bypass, replica_groups=rg,
                        ins=[slice_b[:]], outs=[replica[:]])
                    ctxa.__exit__(None, None, None)
    nc.compile()
    return nc


# ---------------- end-to-end host entry ----------------
_CACHED = {}

def kernel_ex(x, edge_index, edge_weight, W0, b0, convW, W1, b1, trace=False):
    c_ = CFG
    x = np.asarray(x); edge_index = np.asarray(edge_index); edge_weight = np.asarray(edge_weight)
    W0 = np.asarray(W0); convW = np.asarray(convW); W1 = np.asarray(W1)
    b0 = np.asarray(b0); b1 = np.asarray(b1)
    assert np.abs(b0).max() == 0.0 and np.abs(b1).max() == 0.0, "nonzero biases unsupported"
    while True:
        try:
            cores = preprocess(edge_index, edge_weight)
            break
        except OverflowError:
            set_sizes(CFG.N, F_IN=CFG.F_IN, L=CFG.L, main_cols=CFG.MAIN_COLS,
                      ovf_cols=CFG.OVF_COLS + 1)
            _CACHED.pop("nc", None)
    betas = np.log(c_.THETA / np.arange(1, c_.L + 1, dtype=np.float64) + 1.0)
    Wt = np.stack([(1 - bt) * np.eye(c_.H) + bt * Wl.astype(np.float64)
                   for Wl, bt in zip(convW, betas)]).astype(np.float32)
    key = f"nc{int(trace)}"
    if "nc" not in _CACHED:
        _CACHED["nc"] = build_program(debug=_CACHED.get("debug", False))
    nc = _CACHED["nc"]
    in_maps = []
    for c in range(c_.NC):
        xs = np.zeros((c_.NLOC, c_.F_IN), np.float32)
        xs[:c_.NPC] = x[c * c_.NPC:(c + 1) * c_.NPC]
        in_maps.append({
            "x": _bf16(xs), "idx": cores[c]["idx"],
            "Smain": _bf16(cores[c]["Smain"]), "Sovf": _bf16(cores[c]["Sovf"]),
            "W0": _bf16(W0), "Wt": _bf16(Wt), "W1": _bf16(W1),
        })
    res = bass_utils.run_bass_kernel_spmd(nc, in_maps, core_ids=list(range(c_.NC)), trace=trace)
    out = np.concatenate([res.results[c]["out"][:c_.NPC] for c in range(c_.NC)], axis=0)
    return out, res


def kernel(x, edge_index, edge_weight, W0, b0, convW, W1, b1):
    """Harness entry: full inputs in, full [N, C] float32 log-softmax out."""
    out, _ = kernel_ex(x, edge_index, edge_weight, W0, b0, convW, W1, b1, trace=False)
    return out



# revision 10
# speedup vs baseline: 1.0129x; 1.0129x over previous
"""GCN2 (GCNII) forward pass on 8 Trainium2 NeuronCores via Bass/Tile.

Strategy (node sharding per sharding hint):
  - dst nodes sharded across 8 cores; per-core slice padded to NLOC rows.
  - full h replica [NC*NLOC, H] bf16 in DRAM on every core, rebuilt each layer
    via AllGather (the "halo exchange" degenerate case: full replication).
  - aggregation: edges grouped by (dst-group of 512, src-core bucket of NLOC
    rows). Per (group, bucket) ONE dma_gather (int16 window indices) pulls
    MAIN_COLS*4+OVF_COLS columns of 128 messages; selection matrices S
    (precomputed on host, streamed from DRAM) turn segment-sum into PE
    matmuls accumulating aggT[f, 512] in PSUM.
  - epilogue folds the initial-residual term (x0a resident in SBUF), applies
    Wt = (1-beta)I + beta*W via one matmul per 128-node block, relu, writes
    the new h slice; AllGather rebuilds the replica.
  - last layer computes logits + log_softmax on-chip, f32 output.
"""
import math
import numpy as np

import concourse.bass as bass
import concourse.bacc as bacc
import concourse.tile as tile
import concourse.mybir as mybir
import concourse.bass_utils as bass_utils
from concourse.masks import make_identity

# ---------------- configuration ----------------
class CFG:
    NC = 8
    N = 100000
    F_IN = 512
    H = 128
    C = 40
    L = 8
    ALPHA = 0.1
    THETA = 0.5
    MAIN_COLS = 2      # 128-edge columns per (block, bucket)
    OVF_COLS = 1       # group-wide overflow columns per (group, bucket)
    GRP = 512
    BLK = 128

    @classmethod
    def derived(cls):
        cls.NPC = cls.N // cls.NC
        cls.NGRP = math.ceil(cls.NPC / cls.GRP)
        cls.NLOC = cls.NGRP * cls.GRP
        cls.NBLK = cls.GRP // cls.BLK
        cls.NCOLS_M = cls.NBLK * cls.MAIN_COLS
        cls.COLS = cls.NCOLS_M + cls.OVF_COLS
        cls.IDX_GJ = cls.COLS * 128
        cls.KT = cls.F_IN // 128


CFG.derived()


def set_sizes(N, F_IN=512, L=8, main_cols=2, ovf_cols=1):
    CFG.N = N
    CFG.F_IN = F_IN
    CFG.L = L
    CFG.MAIN_COLS = main_cols
    CFG.OVF_COLS = ovf_cols
    CFG.derived()


# ---------------- host preprocessing ----------------
def preprocess(edge_index, edge_weight):
    """Per-core device-feed arrays: wrapped int16 gather indices and dense
    selection matrices (bf16) with (1-alpha)-scaled weights baked in."""
    c_ = CFG
    src = edge_index[0].astype(np.int64)
    dst = edge_index[1].astype(np.int64)
    w = edge_weight.astype(np.float32) * (1.0 - c_.ALPHA)
    cores = []
    for c in range(c_.NC):
        m = (dst // c_.NPC) == c
        s_c, d_c, w_c = src[m], dst[m], w[m]
        dl = d_c - c * c_.NPC
        g = dl // c_.GRP
        b = (dl % c_.GRP) // c_.BLK
        j = s_c // c_.NPC
        ilocal = (s_c % c_.NPC).astype(np.int64)
        slot128 = dl % c_.BLK
        slot512 = dl % c_.GRP
        order = np.lexsort((ilocal, b, j, g))
        g, b, j, ilocal, slot128, slot512, w_c = (a[order] for a in (g, b, j, ilocal, slot128, slot512, w_c))
        key = (g * c_.NC + j)
        idxT = np.zeros((c_.NGRP * c_.NC, c_.IDX_GJ), np.int16)
        Smain = np.zeros((c_.NGRP * c_.NC, 128, c_.NCOLS_M * 128), np.float32)
        Sovf = np.zeros((c_.NGRP * c_.NC, 128, c_.OVF_COLS * c_.GRP), np.float32)
        max_ovf = 0
        # boundaries of (g,j) segments
        seg_start = np.searchsorted(key, np.arange(c_.NGRP * c_.NC), side="left")
        seg_end = np.searchsorted(key, np.arange(c_.NGRP * c_.NC), side="right")
        for gj in range(c_.NGRP * c_.NC):
            s0, s1 = seg_start[gj], seg_end[gj]
            if s0 == s1:
                continue
            bv, iv = b[s0:s1], ilocal[s0:s1]
            s1v, s5v, wv = slot128[s0:s1], slot512[s0:s1], w_c[s0:s1]
            ovf_sel = np.zeros(s1 - s0, bool)
            for bi in range(c_.NBLK):
                bm_idx = np.nonzero(bv == bi)[0]
                cap = c_.MAIN_COLS * 128
                take = bm_idx[:cap]
                base = bi * cap
                nt = len(take)
                idxT[gj, base:base + nt] = iv[take]
                ee = (base + np.arange(nt))
                Smain[gj, ee % 128, (ee // 128) * 128 + s1v[take]] = wv[take]
                if len(bm_idx) > cap:
                    ovf_sel[bm_idx[cap:]] = True
            ov = np.nonzero(ovf_sel)[0]
            ov = ov[np.argsort(iv[ov], kind="stable")]
            no = len(ov)
            max_ovf = max(max_ovf, no)
            if no > c_.OVF_COLS * 128:
                raise OverflowError(f"core {c} gj {gj}: ovf {no} > {c_.OVF_COLS*128}")
            ob = c_.NCOLS_M * 128
            idxT[gj, ob:ob + no] = iv[ov]
            ee = np.arange(no)
            Sovf[gj, ee % 128, (ee // 128) * c_.GRP + s5v[ov]] = wv[ov]
        # wrap idx: flat i -> [i % 16, i // 16], replicated across 8 groups of 16
        wrapped = idxT.reshape(c_.NGRP * c_.NC, c_.IDX_GJ // 16, 16).transpose(0, 2, 1)
        wr = wrapped.transpose(1, 0, 2).reshape(16, -1)
        idx_dev = np.zeros((128, c_.NGRP * c_.NC * (c_.IDX_GJ // 16)), np.int16)
        for q in range(8):
            idx_dev[16 * q:16 * q + 16, :] = wr
        cores.append(dict(idx=idx_dev, Smain=Smain, Sovf=Sovf, max_ovf=max_ovf))
    return cores


def _bf16(a):
    import ml_dtypes
    return np.asarray(a, dtype=ml_dtypes.bfloat16)


# ---------------- device program ----------------
def build_program(debug=False):
    c_ = CFG
    nc = bacc.Bacc("TRN2", target_bir_lowering=False, debug=False,
                   enable_asserts=True, num_devices=c_.NC, num_swdge_queues=4)
    bf = mybir.dt.bfloat16
    f32 = mybir.dt.float32
    x_t = nc.dram_tensor("x", [c_.NLOC, c_.F_IN], bf, kind="ExternalInput").ap()
    idx_t = nc.dram_tensor("idx", [128, c_.NGRP * c_.NC * (c_.IDX_GJ // 16)], mybir.dt.int16, kind="ExternalInput").ap()
    sm_t = nc.dram_tensor("Smain", [c_.NGRP * c_.NC, 128, c_.NCOLS_M * 128], bf, kind="ExternalInput").ap()
    so_t = nc.dram_tensor("Sovf", [c_.NGRP * c_.NC, 128, c_.OVF_COLS * c_.GRP], bf, kind="ExternalInput").ap()
    w0_t = nc.dram_tensor("W0", [c_.F_IN, c_.H], bf, kind="ExternalInput").ap()
    wt_t = nc.dram_tensor("Wt", [c_.L, c_.H, c_.H], bf, kind="ExternalInput").ap()
    w1_t = nc.dram_tensor("W1", [c_.H, c_.C], bf, kind="ExternalInput").ap()
    out_t = nc.dram_tensor("out", [c_.NLOC, c_.C], f32, kind="ExternalOutput").ap()
    if debug:
        dbg_h0 = nc.dram_tensor("dbg_h0", [c_.NLOC, c_.H], bf, kind="ExternalOutput").ap()
        dbg_rep = nc.dram_tensor("dbg_rep", [c_.NC * c_.NLOC, c_.H], bf, kind="ExternalOutput").ap()
        dbg_outT = nc.dram_tensor("dbg_outT", [128, c_.GRP], f32, kind="ExternalOutput").ap()
        dbg_gt = nc.dram_tensor("dbg_gt", [128, c_.COLS * 128], bf, kind="ExternalOutput").ap()

    slice_b = nc.dram_tensor("slice_b", [c_.NLOC, c_.H], bf, kind="Internal").ap()
    replica = nc.dram_tensor("replica", [c_.NC * c_.NLOC, c_.H], bf, kind="Internal", addr_space="Shared").ap()

    rg = [list(range(c_.NC))]

    with tile.TileContext(nc) as tc:
        with tc.tile_pool(name="res", bufs=1) as res, \
             tc.tile_pool(name="work", bufs=1) as work, \
             tc.tile_pool(name="psum", bufs=1, space="PSUM") as psum:
            # resident tensors
            idx = res.tile([128, c_.NGRP * c_.NC * (c_.IDX_GJ // 16)], mybir.dt.int16)
            nc.sync.dma_start(idx[:], idx_t[:])
            w0 = res.tile([128, c_.KT, c_.H], bf)
            nc.sync.dma_start(w0[:], w0_t[:].rearrange("(k p) f -> p k f", p=128))
            wt = res.tile([128, c_.L, c_.H], bf)
            nc.sync.dma_start(wt[:], wt_t[:].rearrange("l f j -> f l j"))
            w1 = res.tile([128, c_.C], bf)
            nc.sync.dma_start(w1[:], w1_t[:])
            ident = res.tile([128, 128], bf)
            make_identity(nc, ident[:])
            x0a = res.tile([128, c_.NLOC], bf)

            # ---------- layer 0: h0 = relu(x @ W0); x0a = alpha*h0 ----------
            ctx0 = nc.named_scope("layer0"); ctx0.__enter__()
            for g in range(c_.NGRP):
                ph0 = psum.tile([128, c_.GRP], f32, name=f"pga{g%2}", tag="pga", bufs=2, space="PSUM")
                for k in range(c_.KT):
                    xt = work.tile([128, c_.GRP], bf, name=f"xt{k%3}", tag="xt", bufs=3)
                    nc.sync.dma_start(out=xt[:], in_=x_t[g * c_.GRP:(g + 1) * c_.GRP, k * 128:(k + 1) * 128], transpose=True)
                    nc.tensor.matmul(out=ph0[:], lhsT=w0[:, k, :], rhs=xt[:],
                                     start=(k == 0), stop=(k == c_.KT - 1))
                nc.scalar.activation(out=x0a[:, g * c_.GRP:(g + 1) * c_.GRP], in_=ph0[:],
                                     func=mybir.ActivationFunctionType.Relu, scale=c_.ALPHA)
                h0T = work.tile([128, c_.GRP], bf, name=f"h0T{g%2}", tag="h0T", bufs=2)
                nc.scalar.activation(out=h0T[:], in_=ph0[:], func=mybir.ActivationFunctionType.Relu)
                for b in range(c_.NBLK):
                    ptr = psum.tile([128, 128], bf, name=f"ptr{b%2}", tag="p2", bufs=4, space="PSUM")
                    nc.tensor.transpose(out=ptr[:], in_=h0T[:, b * 128:(b + 1) * 128], identity=ident[:])
                    hrow = work.tile([128, 128], bf, name=f"hrow{b%2}", tag="hrow", bufs=4)
                    nc.vector.tensor_copy(out=hrow[:], in_=ptr[:])
                    nc.sync.dma_start(out=slice_b[g * c_.GRP + b * 128: g * c_.GRP + (b + 1) * 128, :], in_=hrow[:])
            ctx0.__exit__(None, None, None)
            ctxag = nc.named_scope("ag0"); ctxag.__enter__()
            nc.gpsimd.collective_compute(
                "AllGather", mybir.AluOpType.bypass, replica_groups=rg,
                ins=[slice_b[:]], outs=[replica[:]])
            ctxag.__exit__(None, None, None)
            if debug:
                dcp = work.tile([128, c_.H], bf, name="dcp", tag="hrow", bufs=4)
                for r in range(c_.NLOC // 128):
                    nc.sync.dma_start(out=dcp[:], in_=slice_b[r * 128:(r + 1) * 128, :])
                    nc.scalar.dma_start(out=dbg_h0[r * 128:(r + 1) * 128, :], in_=dcp[:])
                for r in range(c_.NC * c_.NLOC // 128):
                    nc.sync.dma_start(out=dcp[:], in_=replica[r * 128:(r + 1) * 128, :])
                    nc.scalar.dma_start(out=dbg_rep[r * 128:(r + 1) * 128, :], in_=dcp[:])

            # ---------- conv layers ----------
            for l in range(c_.L):
                last = (l == c_.L - 1)
                ctxl = nc.named_scope(f"conv{l}"); ctxl.__enter__()
                for g in range(c_.NGRP):
                    pga = psum.tile([128, c_.GRP], f32, name=f"pga{g%2}", tag="pga", bufs=2, space="PSUM")
                    pgb = psum.tile([128, c_.GRP], f32, name=f"pgb{g%2}", tag="pgb", bufs=2, space="PSUM")
                    nc.tensor.matmul(out=pga[:], lhsT=ident[:],
                                     rhs=x0a[:, g * c_.GRP:(g + 1) * c_.GRP],
                                     start=True, stop=False, skip_group_check=True)
                    for j in range(c_.NC):
                        gj = g * c_.NC + j
                        pagg = pga if j < 4 else pgb
                        gt = work.tile([128, c_.COLS, 128], bf, name=f"gt{j%4}", tag="gt", bufs=10)
                        nc.gpsimd.dma_gather(
                            out_ap=gt[:], in_ap=replica[j * c_.NLOC:(j + 1) * c_.NLOC, :],
                            idxs_ap=idx[:, gj * (c_.IDX_GJ // 16):(gj + 1) * (c_.IDX_GJ // 16)],
                            num_idxs=c_.IDX_GJ, num_idxs_reg=c_.IDX_GJ, elem_size=c_.H,
                            queue_num=j % 4, single_packet=False)
                        sm = work.tile([128, c_.NCOLS_M * 128], bf, name=f"sm{j%3}", tag="sm", bufs=8)
                        nc.sync.dma_start(out=sm[:], in_=sm_t[gj, :, :])
                        so = work.tile([128, c_.OVF_COLS * c_.GRP], bf, name=f"so{j%3}", tag="so", bufs=8)
                        nc.scalar.dma_start(out=so[:], in_=so_t[gj, :, :])
                        # j==4: overflow column first; its start=True (512-wide,
                        # covers the whole bank) resets bank B - no init matmul.
                        if j == 4:
                            for oc in range(c_.OVF_COLS):
                                nc.tensor.matmul(out=pagg[:], lhsT=gt[:, c_.NCOLS_M + oc, :],
                                                 rhs=so[:, oc * c_.GRP:(oc + 1) * c_.GRP],
                                                 start=(oc == 0), stop=False,
                                                 skip_group_check=True)
                        for cc in range(c_.NCOLS_M):
                            bi = cc // c_.MAIN_COLS
                            nc.tensor.matmul(out=pagg[:, bi * 128:(bi + 1) * 128],
                                             lhsT=gt[:, cc, :], rhs=sm[:, cc * 128:(cc + 1) * 128],
                                             start=False, stop=False, skip_group_check=True)
                        if j != 4:
                            for oc in range(c_.OVF_COLS):
                                nc.tensor.matmul(out=pagg[:], lhsT=gt[:, c_.NCOLS_M + oc, :],
                                                 rhs=so[:, oc * c_.GRP:(oc + 1) * c_.GRP],
                                                 start=False,
                                                 stop=((j == 3 or j == c_.NC - 1) and oc == c_.OVF_COLS - 1),
                                                 skip_group_check=True)
                    aggb = work.tile([128, c_.GRP], bf, name=f"aggb{g%2}", tag="aggb", bufs=3)
                    nc.scalar.activation(out=aggb[:], in_=pgb[:], func=mybir.ActivationFunctionType.Copy)
                    outT = work.tile([128, c_.GRP], bf, name=f"outT{g%2}", tag="outT", bufs=3)
                    nc.vector.tensor_tensor(out=outT[:], in0=pga[:], in1=aggb[:],
                                            op=mybir.AluOpType.add)
                    for b in range(c_.NBLK):
                        if not last:
                            p2 = psum.tile([128, 128], f32, name=f"p2{b%3}", tag="p2", bufs=4, space="PSUM")
                            nc.tensor.matmul(out=p2[:], lhsT=outT[:, b * 128:(b + 1) * 128],
                                             rhs=wt[:, l, :], start=True, stop=True)
                            hnew = work.tile([128, 128], bf, name=f"hnew{b%2}", tag="hrow", bufs=4)
                            nc.scalar.activation(out=hnew[:], in_=p2[:], func=mybir.ActivationFunctionType.Relu)
                            eng = nc.sync if b % 2 == 0 else nc.scalar
                            eng.dma_start(out=slice_b[g * c_.GRP + b * 128: g * c_.GRP + (b + 1) * 128, :], in_=hnew[:])
                        else:
                            p2 = psum.tile([128, 128], f32, name=f"p2{b%3}", tag="p2", bufs=4, space="PSUM")
                            nc.tensor.matmul(out=p2[:], lhsT=wt[:, l, :],
                                             rhs=outT[:, b * 128:(b + 1) * 128], start=True, stop=True)
                            h8T = work.tile([128, 128], bf, name=f"h8T{b%2}", tag="hrow", bufs=4)
                            nc.scalar.activation(out=h8T[:], in_=p2[:], func=mybir.ActivationFunctionType.Relu)
                            plg = psum.tile([128, 128], f32, name=f"plg{b%3}", tag="p2", bufs=4, space="PSUM")
                            nc.tensor.matmul(out=plg[:, :c_.C], lhsT=h8T[:], rhs=w1[:], start=True, stop=True)
                            negm = work.tile([128, 1], f32, name=f"negm{b%2}", tag="negm", bufs=4)
                            nc.vector.reduce_max(out=negm[:], in_=plg[:, :c_.C], axis=mybir.AxisListType.X, negate=True)
                            esc = work.tile([128, c_.C], bf, name=f"esc{b%2}", tag="esc", bufs=2)
                            ssum = work.tile([128, 1], f32, name=f"ssum{b%2}", tag="ssum", bufs=4)
                            nc.scalar.activation(out=esc[:], in_=plg[:, :c_.C], func=mybir.ActivationFunctionType.Exp,
                                                 bias=negm[:, :1], accum_out=ssum[:, :1])
                            lsum = work.tile([128, 1], f32, name=f"lsum{b%2}", tag="lsum", bufs=4)
                            nc.scalar.activation(out=lsum[:], in_=ssum[:], func=mybir.ActivationFunctionType.Ln)
                            fin = work.tile([128, c_.C], f32, name=f"fin{b%2}", tag="fin", bufs=4)
                            nc.vector.tensor_scalar(out=fin[:], in0=plg[:, :c_.C],
                                                    scalar1=negm[:, :1], scalar2=lsum[:, :1],
                                                    op0=mybir.AluOpType.add, op1=mybir.AluOpType.subtract)
                            eng = nc.sync if b % 2 == 0 else nc.scalar
                            eng.dma_start(out=out_t[g * c_.GRP + b * 128: g * c_.GRP + (b + 1) * 128, :], in_=fin[:])
                ctxl.__exit__(None, None, None)
                if not last:
                    ctxa = nc.named_scope(f"ag{l+1}"); ctxa.__enter__()
                    nc.gpsimd.collective_compute(
                        "AllGather", mybir.AluOpType.bypass, replica_groups=rg,
                        ins=[slice_b[:]], outs=[replica[:]])
                    ctxa.__exit__(None, None, None)
    nc.compile()
    return nc


# ---------------- end-to-end host entry ----------------
_CACHED = {}

def kernel_ex(x, edge_index, edge_weight, W0, b0, convW, W1, b1, trace=False):
    c_ = CFG
    x = np.asarray(x); edge_index = np.asarray(edge_index); edge_weight = np.asarray(edge_weight)
    W0 = np.asarray(W0); convW = np.asarray(convW); W1 = np.asarray(W1)
    b0 = np.asarray(b0); b1 = np.asarray(b1)
    assert np.abs(b0).max() == 0.0 and np.abs(b1).max() == 0.0, "nonzero biases unsupported"
    while True:
        try:
            cores = preprocess(edge_index, edge_weight)
            break
        except OverflowError:
            set_sizes(CFG.N, F_IN=CFG.F_IN, L=CFG.L, main_cols=CFG.MAIN_COLS,
                      ovf_cols=CFG.OVF_COLS + 1)
            _CACHED.pop("nc", None)
    betas = np.log(c_.THETA / np.arange(1, c_.L + 1, dtype=np.float64) + 1.0)
    Wt = np.stack([(1 - bt) * np.eye(c_.H) + bt * Wl.astype(np.float64)
                   for Wl, bt in zip(convW, betas)]).astype(np.float32)
    key = f"nc{int(trace)}"
    if "nc" not in _CACHED:
        _CACHED["nc"] = build_program(debug=_CACHED.get("debug", False))
    nc = _CACHED["nc"]
    in_maps = []
    for c in range(c_.NC):
        xs = np.zeros((c_.NLOC, c_.F_IN), np.float32)
        xs[:c_.NPC] = x[c * c_.NPC:(c + 1) * c_.NPC]
        in_maps.append({
            "x": _bf16(xs), "idx": cores[c]["idx"],
            "Smain": _bf16(cores[c]["Smain"]), "Sovf": _bf16(cores[c]["Sovf"]),
            "W0": _bf16(W0), "Wt": _bf16(Wt), "W1": _bf16(W1),
        })
    res = bass_utils.run_bass_kernel_spmd(nc, in_maps, core_ids=list(range(c_.NC)), trace=trace)
    out = np.concatenate([res.results[c]["out"][:c_.NPC] for c in range(c_.NC)], axis=0)
    return out, res


def kernel(x, edge_index, edge_weight, W0, b0, convW, W1, b1):
    """Harness entry: full inputs in, full [N, C] float32 log-softmax out."""
    out, _ = kernel_ex(x, edge_index, edge_weight, W0, b0, convW, W1, b1, trace=False)
    return out



# revision 11
# speedup vs baseline: 1.0819x; 1.0682x over previous
"""GCN2 (GCNII) forward pass on 8 Trainium2 NeuronCores via Bass/Tile.

Strategy (node sharding per sharding hint):
  - dst nodes sharded across 8 cores; per-core slice padded to NLOC rows.
  - full h replica [NC*NLOC, H] bf16 in DRAM on every core, rebuilt each layer
    via AllGather (the "halo exchange" degenerate case: full replication).
  - aggregation: edges grouped by (dst-group of 512, src-core bucket of NLOC
    rows). Per (group, bucket) ONE dma_gather (int16 window indices) pulls
    MAIN_COLS*4+OVF_COLS columns of 128 messages; selection matrices S
    (precomputed on host, streamed from DRAM) turn segment-sum into PE
    matmuls accumulating aggT[f, 512] in PSUM.
  - epilogue folds the initial-residual term (x0a resident in SBUF), applies
    Wt = (1-beta)I + beta*W via one matmul per 128-node block, relu, writes
    the new h slice; AllGather rebuilds the replica.
  - last layer computes logits + log_softmax on-chip, f32 output.
"""
import math
import numpy as np

import concourse.bass as bass
import concourse.bacc as bacc
import concourse.tile as tile
import concourse.mybir as mybir
import concourse.bass_utils as bass_utils
from concourse.masks import make_identity

# ---------------- configuration ----------------
class CFG:
    NC = 8
    N = 100000
    F_IN = 512
    H = 128
    C = 40
    L = 8
    ALPHA = 0.1
    THETA = 0.5
    MAIN_COLS = 2      # 128-edge columns per (block, bucket)
    OVF_COLS = 1       # group-wide overflow columns per (group, bucket)
    GRP = 512
    BLK = 128

    @classmethod
    def derived(cls):
        cls.NPC = cls.N // cls.NC
        cls.NGRP = math.ceil(cls.NPC / cls.GRP)
        cls.NLOC = cls.NGRP * cls.GRP
        cls.NBLK = cls.GRP // cls.BLK
        cls.NCOLS_M = cls.NBLK * cls.MAIN_COLS
        cls.COLS = cls.NCOLS_M + cls.OVF_COLS
        cls.IDX_GJ = cls.COLS * 128
        cls.KT = cls.F_IN // 128


CFG.derived()


def set_sizes(N, F_IN=512, L=8, main_cols=2, ovf_cols=1):
    CFG.N = N
    CFG.F_IN = F_IN
    CFG.L = L
    CFG.MAIN_COLS = main_cols
    CFG.OVF_COLS = ovf_cols
    CFG.derived()


# ---------------- host preprocessing ----------------
def preprocess(edge_index, edge_weight):
    """Per-core device-feed arrays: wrapped int16 gather indices and dense
    selection matrices (bf16) with (1-alpha)-scaled weights baked in."""
    c_ = CFG
    src = edge_index[0].astype(np.int64)
    dst = edge_index[1].astype(np.int64)
    w = edge_weight.astype(np.float32) * (1.0 - c_.ALPHA)
    cores = []
    for c in range(c_.NC):
        m = (dst // c_.NPC) == c
        s_c, d_c, w_c = src[m], dst[m], w[m]
        dl = d_c - c * c_.NPC
        g = dl // c_.GRP
        b = (dl % c_.GRP) // c_.BLK
        j = s_c // c_.NPC
        ilocal = (s_c % c_.NPC).astype(np.int64)
        slot128 = dl % c_.BLK
        slot512 = dl % c_.GRP
        order = np.lexsort((ilocal, b, j, g))
        g, b, j, ilocal, slot128, slot512, w_c = (a[order] for a in (g, b, j, ilocal, slot128, slot512, w_c))
        key = (g * c_.NC + j)
        idxT = np.zeros((c_.NGRP * c_.NC, c_.IDX_GJ), np.int16)
        Smain = np.zeros((c_.NGRP * c_.NC, 128, c_.NCOLS_M * 128), np.float32)
        movf = np.zeros((c_.NGRP * c_.NC, 128, 2 * c_.OVF_COLS), np.float32)
        max_ovf = 0
        # boundaries of (g,j) segments
        seg_start = np.searchsorted(key, np.arange(c_.NGRP * c_.NC), side="left")
        seg_end = np.searchsorted(key, np.arange(c_.NGRP * c_.NC), side="right")
        for gj in range(c_.NGRP * c_.NC):
            s0, s1 = seg_start[gj], seg_end[gj]
            if s0 == s1:
                continue
            bv, iv = b[s0:s1], ilocal[s0:s1]
            s1v, s5v, wv = slot128[s0:s1], slot512[s0:s1], w_c[s0:s1]
            ovf_sel = np.zeros(s1 - s0, bool)
            for bi in range(c_.NBLK):
                bm_idx = np.nonzero(bv == bi)[0]
                cap = c_.MAIN_COLS * 128
                take = bm_idx[:cap]
                base = bi * cap
                nt = len(take)
                idxT[gj, base:base + nt] = iv[take]
                ee = (base + np.arange(nt))
                Smain[gj, ee % 128, (ee // 128) * 128 + s1v[take]] = wv[take]
                if len(bm_idx) > cap:
                    ovf_sel[bm_idx[cap:]] = True
            ov = np.nonzero(ovf_sel)[0]
            ov = ov[np.argsort(iv[ov], kind="stable")]
            no = len(ov)
            max_ovf = max(max_ovf, no)
            if no > c_.OVF_COLS * 128:
                raise OverflowError(f"core {c} gj {gj}: ovf {no} > {c_.OVF_COLS*128}")
            ob = c_.NCOLS_M * 128
            idxT[gj, ob:ob + no] = iv[ov]
            ee = np.arange(no)
            movf[gj, ee % 128, 2 * (ee // 128)] = s5v[ov]
            movf[gj, ee % 128, 2 * (ee // 128) + 1] = wv[ov]
        # wrap idx: flat i -> [i % 16, i // 16], replicated across 8 groups of 16
        wrapped = idxT.reshape(c_.NGRP * c_.NC, c_.IDX_GJ // 16, 16).transpose(0, 2, 1)
        wr = wrapped.transpose(1, 0, 2).reshape(16, -1)
        idx_dev = np.zeros((128, c_.NGRP * c_.NC * (c_.IDX_GJ // 16)), np.int16)
        for q in range(8):
            idx_dev[16 * q:16 * q + 16, :] = wr
        cores.append(dict(idx=idx_dev, Smain=Smain, movf=movf, max_ovf=max_ovf))
    return cores


def _bf16(a):
    import ml_dtypes
    return np.asarray(a, dtype=ml_dtypes.bfloat16)


# ---------------- device program ----------------
def build_program(debug=False):
    c_ = CFG
    nc = bacc.Bacc("TRN2", target_bir_lowering=False, debug=False,
                   enable_asserts=True, num_devices=c_.NC, num_swdge_queues=4)
    bf = mybir.dt.bfloat16
    f32 = mybir.dt.float32
    x_t = nc.dram_tensor("x", [c_.NLOC, c_.F_IN], bf, kind="ExternalInput").ap()
    idx_t = nc.dram_tensor("idx", [128, c_.NGRP * c_.NC * (c_.IDX_GJ // 16)], mybir.dt.int16, kind="ExternalInput").ap()
    sm_t = nc.dram_tensor("Smain", [c_.NGRP * c_.NC, 128, c_.NCOLS_M * 128], bf, kind="ExternalInput").ap()
    mo_t = nc.dram_tensor("movf", [c_.NGRP * c_.NC, 128, 2 * c_.OVF_COLS], f32, kind="ExternalInput").ap()
    w0_t = nc.dram_tensor("W0", [c_.F_IN, c_.H], bf, kind="ExternalInput").ap()
    wt_t = nc.dram_tensor("Wt", [c_.L, c_.H, c_.H], bf, kind="ExternalInput").ap()
    w1_t = nc.dram_tensor("W1", [c_.H, c_.C], bf, kind="ExternalInput").ap()
    out_t = nc.dram_tensor("out", [c_.NLOC, c_.C], f32, kind="ExternalOutput").ap()
    if debug:
        dbg_h0 = nc.dram_tensor("dbg_h0", [c_.NLOC, c_.H], bf, kind="ExternalOutput").ap()
        dbg_rep = nc.dram_tensor("dbg_rep", [c_.NC * c_.NLOC, c_.H], bf, kind="ExternalOutput").ap()
        dbg_outT = nc.dram_tensor("dbg_outT", [128, c_.GRP], f32, kind="ExternalOutput").ap()
        dbg_gt = nc.dram_tensor("dbg_gt", [128, c_.COLS * 128], bf, kind="ExternalOutput").ap()

    slice_b = nc.dram_tensor("slice_b", [c_.NLOC, c_.H], bf, kind="Internal").ap()
    replica = nc.dram_tensor("replica", [c_.NC * c_.NLOC, c_.H], bf, kind="Internal", addr_space="Shared").ap()

    rg = [list(range(c_.NC))]

    with tile.TileContext(nc) as tc:
        with tc.tile_pool(name="res", bufs=1) as res, \
             tc.tile_pool(name="work", bufs=1) as work, \
             tc.tile_pool(name="psum", bufs=1, space="PSUM") as psum:
            # resident tensors
            idx = res.tile([128, c_.NGRP * c_.NC * (c_.IDX_GJ // 16)], mybir.dt.int16)
            nc.sync.dma_start(idx[:], idx_t[:])
            w0 = res.tile([128, c_.KT, c_.H], bf)
            nc.sync.dma_start(w0[:], w0_t[:].rearrange("(k p) f -> p k f", p=128))
            wt = res.tile([128, c_.L, c_.H], bf)
            nc.sync.dma_start(wt[:], wt_t[:].rearrange("l f j -> f l j"))
            w1 = res.tile([128, c_.C], bf)
            nc.sync.dma_start(w1[:], w1_t[:])
            ident = res.tile([128, 128], bf)
            make_identity(nc, ident[:])
            iota = res.tile([128, c_.GRP], f32)
            nc.gpsimd.iota(iota[:], pattern=[[1, c_.GRP]], base=0,
                           channel_multiplier=0,
                           allow_small_or_imprecise_dtypes=True)
            x0a = res.tile([128, c_.NLOC], bf)

            # ---------- layer 0: h0 = relu(x @ W0); x0a = alpha*h0 ----------
            ctx0 = nc.named_scope("layer0"); ctx0.__enter__()
            for g in range(c_.NGRP):
                ph0 = psum.tile([128, c_.GRP], f32, name=f"pga{g%2}", tag="pga", bufs=2, space="PSUM")
                for k in range(c_.KT):
                    xt = work.tile([128, c_.GRP], bf, name=f"xt{k%3}", tag="xt", bufs=3)
                    nc.sync.dma_start(out=xt[:], in_=x_t[g * c_.GRP:(g + 1) * c_.GRP, k * 128:(k + 1) * 128], transpose=True)
                    nc.tensor.matmul(out=ph0[:], lhsT=w0[:, k, :], rhs=xt[:],
                                     start=(k == 0), stop=(k == c_.KT - 1))
                nc.scalar.activation(out=x0a[:, g * c_.GRP:(g + 1) * c_.GRP], in_=ph0[:],
                                     func=mybir.ActivationFunctionType.Relu, scale=c_.ALPHA)
                h0T = work.tile([128, c_.GRP], bf, name=f"h0T{g%2}", tag="h0T", bufs=2)
                nc.scalar.activation(out=h0T[:], in_=ph0[:], func=mybir.ActivationFunctionType.Relu)
                for b in range(c_.NBLK):
                    ptr = psum.tile([128, 128], bf, name=f"ptr{b%2}", tag="p2", bufs=4, space="PSUM")
                    nc.tensor.transpose(out=ptr[:], in_=h0T[:, b * 128:(b + 1) * 128], identity=ident[:])
                    hrow = work.tile([128, 128], bf, name=f"hrow{b%2}", tag="hrow", bufs=4)
                    nc.vector.tensor_copy(out=hrow[:], in_=ptr[:])
                    nc.sync.dma_start(out=slice_b[g * c_.GRP + b * 128: g * c_.GRP + (b + 1) * 128, :], in_=hrow[:])
            ctx0.__exit__(None, None, None)
            ctxag = nc.named_scope("ag0"); ctxag.__enter__()
            nc.gpsimd.collective_compute(
                "AllGather", mybir.AluOpType.bypass, replica_groups=rg,
                ins=[slice_b[:]], outs=[replica[:]])
            ctxag.__exit__(None, None, None)
            if debug:
                dcp = work.tile([128, c_.H], bf, name="dcp", tag="hrow", bufs=4)
                for r in range(c_.NLOC // 128):
                    nc.sync.dma_start(out=dcp[:], in_=slice_b[r * 128:(r + 1) * 128, :])
                    nc.scalar.dma_start(out=dbg_h0[r * 128:(r + 1) * 128, :], in_=dcp[:])
                for r in range(c_.NC * c_.NLOC // 128):
                    nc.sync.dma_start(out=dcp[:], in_=replica[r * 128:(r + 1) * 128, :])
                    nc.scalar.dma_start(out=dbg_rep[r * 128:(r + 1) * 128, :], in_=dcp[:])

            # ---------- conv layers ----------
            for l in range(c_.L):
                last = (l == c_.L - 1)
                ctxl = nc.named_scope(f"conv{l}"); ctxl.__enter__()
                for g in range(c_.NGRP):
                    pga = psum.tile([128, c_.GRP], f32, name=f"pga{g%2}", tag="pga", bufs=2, space="PSUM")
                    pgb = psum.tile([128, c_.GRP], f32, name=f"pgb{g%2}", tag="pgb", bufs=2, space="PSUM")
                    nc.tensor.matmul(out=pga[:], lhsT=ident[:],
                                     rhs=x0a[:, g * c_.GRP:(g + 1) * c_.GRP],
                                     start=True, stop=False, skip_group_check=True)
                    for j in range(c_.NC):
                        gj = g * c_.NC + j
                        pagg = pga if j < 4 else pgb
                        gt = work.tile([128, c_.COLS, 128], bf, name=f"gt{j%4}", tag="gt", bufs=10)
                        nc.gpsimd.dma_gather(
                            out_ap=gt[:], in_ap=replica[j * c_.NLOC:(j + 1) * c_.NLOC, :],
                            idxs_ap=idx[:, gj * (c_.IDX_GJ // 16):(gj + 1) * (c_.IDX_GJ // 16)],
                            num_idxs=c_.IDX_GJ, num_idxs_reg=c_.IDX_GJ, elem_size=c_.H,
                            queue_num=j % 4, single_packet=False)
                        sm = work.tile([128, c_.NCOLS_M * 128], bf, name=f"sm{j%3}", tag="sm", bufs=8)
                        nc.sync.dma_start(out=sm[:], in_=sm_t[gj, :, :])
                        mo = work.tile([128, 2 * c_.OVF_COLS], f32, name=f"mo{j%4}", tag="mo", bufs=8)
                        nc.scalar.dma_start(out=mo[:], in_=mo_t[gj, :, :])
                        so = work.tile([128, c_.OVF_COLS * c_.GRP], bf, name=f"so{j%3}", tag="so", bufs=4)
                        for oc in range(c_.OVF_COLS):
                            nc.vector.tensor_scalar(
                                out=so[:, oc * c_.GRP:(oc + 1) * c_.GRP], in0=iota[:],
                                scalar1=mo[:, 2 * oc:2 * oc + 1],
                                scalar2=mo[:, 2 * oc + 1:2 * oc + 2],
                                op0=mybir.AluOpType.is_equal, op1=mybir.AluOpType.mult)
                        # j==4: overflow column first; its start=True (512-wide,
                        # covers the whole bank) resets bank B - no init matmul.
                        if j == 4:
                            for oc in range(c_.OVF_COLS):
                                nc.tensor.matmul(out=pagg[:], lhsT=gt[:, c_.NCOLS_M + oc, :],
                                                 rhs=so[:, oc * c_.GRP:(oc + 1) * c_.GRP],
                                                 start=(oc == 0), stop=False,
                                                 skip_group_check=True)
                        for cc in range(c_.NCOLS_M):
                            bi = cc // c_.MAIN_COLS
                            nc.tensor.matmul(out=pagg[:, bi * 128:(bi + 1) * 128],
                                             lhsT=gt[:, cc, :], rhs=sm[:, cc * 128:(cc + 1) * 128],
                                             start=False, stop=False, skip_group_check=True)
                        if j != 4:
                            for oc in range(c_.OVF_COLS):
                                nc.tensor.matmul(out=pagg[:], lhsT=gt[:, c_.NCOLS_M + oc, :],
                                                 rhs=so[:, oc * c_.GRP:(oc + 1) * c_.GRP],
                                                 start=False,
                                                 stop=((j == 3 or j == c_.NC - 1) and oc == c_.OVF_COLS - 1),
                                                 skip_group_check=True)
                    aggb = work.tile([128, c_.GRP], bf, name=f"aggb{g%2}", tag="aggb", bufs=3)
                    nc.scalar.activation(out=aggb[:], in_=pgb[:], func=mybir.ActivationFunctionType.Copy)
                    outT = work.tile([128, c_.GRP], bf, name=f"outT{g%2}", tag="outT", bufs=3)
                    nc.vector.tensor_tensor(out=outT[:], in0=pga[:], in1=aggb[:],
                                            op=mybir.AluOpType.add)
                    for b in range(c_.NBLK):
                        if not last:
                            p2 = psum.tile([128, 128], f32, name=f"p2{b%3}", tag="p2", bufs=4, space="PSUM")
                            nc.tensor.matmul(out=p2[:], lhsT=outT[:, b * 128:(b + 1) * 128],
                                             rhs=wt[:, l, :], start=True, stop=True)
                            hnew = work.tile([128, 128], bf, name=f"hnew{b%2}", tag="hrow", bufs=4)
                            nc.scalar.activation(out=hnew[:], in_=p2[:], func=mybir.ActivationFunctionType.Relu)
                            eng = nc.sync if b % 2 == 0 else nc.scalar
                            eng.dma_start(out=slice_b[g * c_.GRP + b * 128: g * c_.GRP + (b + 1) * 128, :], in_=hnew[:])
                        else:
                            p2 = psum.tile([128, 128], f32, name=f"p2{b%3}", tag="p2", bufs=4, space="PSUM")
                            nc.tensor.matmul(out=p2[:], lhsT=wt[:, l, :],
                                             rhs=outT[:, b * 128:(b + 1) * 128], start=True, stop=True)
                            h8T = work.tile([128, 128], bf, name=f"h8T{b%2}", tag="hrow", bufs=4)
                            nc.scalar.activation(out=h8T[:], in_=p2[:], func=mybir.ActivationFunctionType.Relu)
                            plg = psum.tile([128, 128], f32, name=f"plg{b%3}", tag="p2", bufs=4, space="PSUM")
                            nc.tensor.matmul(out=plg[:, :c_.C], lhsT=h8T[:], rhs=w1[:], start=True, stop=True)
                            negm = work.tile([128, 1], f32, name=f"negm{b%2}", tag="negm", bufs=4)
                            nc.vector.reduce_max(out=negm[:], in_=plg[:, :c_.C], axis=mybir.AxisListType.X, negate=True)
                            esc = work.tile([128, c_.C], bf, name=f"esc{b%2}", tag="esc", bufs=2)
                            ssum = work.tile([128, 1], f32, name=f"ssum{b%2}", tag="ssum", bufs=4)
                            nc.scalar.activation(out=esc[:], in_=plg[:, :c_.C], func=mybir.ActivationFunctionType.Exp,
                                                 bias=negm[:, :1], accum_out=ssum[:, :1])
                            lsum = work.tile([128, 1], f32, name=f"lsum{b%2}", tag="lsum", bufs=4)
                            nc.scalar.activation(out=lsum[:], in_=ssum[:], func=mybir.ActivationFunctionType.Ln)
                            fin = work.tile([128, c_.C], f32, name=f"fin{b%2}", tag="fin", bufs=4)
                            nc.vector.tensor_scalar(out=fin[:], in0=plg[:, :c_.C],
                                                    scalar1=negm[:, :1], scalar2=lsum[:, :1],
                                                    op0=mybir.AluOpType.add, op1=mybir.AluOpType.subtract)
                            eng = nc.sync if b % 2 == 0 else nc.scalar
                            eng.dma_start(out=out_t[g * c_.GRP + b * 128: g * c_.GRP + (b + 1) * 128, :], in_=fin[:])
                ctxl.__exit__(None, None, None)
                if not last:
                    ctxa = nc.named_scope(f"ag{l+1}"); ctxa.__enter__()
                    nc.gpsimd.collective_compute(
                        "AllGather", mybir.AluOpType.bypass, replica_groups=rg,
                        ins=[slice_b[:]], outs=[replica[:]])
                    ctxa.__exit__(None, None, None)
    nc.compile()
    return nc


# ---------------- end-to-end host entry ----------------
_CACHED = {}

def kernel_ex(x, edge_index, edge_weight, W0, b0, convW, W1, b1, trace=False):
    c_ = CFG
    x = np.asarray(x); edge_index = np.asarray(edge_index); edge_weight = np.asarray(edge_weight)
    W0 = np.asarray(W0); convW = np.asarray(convW); W1 = np.asarray(W1)
    b0 = np.asarray(b0); b1 = np.asarray(b1)
    assert np.abs(b0).max() == 0.0 and np.abs(b1).max() == 0.0, "nonzero biases unsupported"
    while True:
        try:
            cores = preprocess(edge_index, edge_weight)
            break
        except OverflowError:
            set_sizes(CFG.N, F_IN=CFG.F_IN, L=CFG.L, main_cols=CFG.MAIN_COLS,
                      ovf_cols=CFG.OVF_COLS + 1)
            _CACHED.pop("nc", None)
    betas = np.log(c_.THETA / np.arange(1, c_.L + 1, dtype=np.float64) + 1.0)
    Wt = np.stack([(1 - bt) * np.eye(c_.H) + bt * Wl.astype(np.float64)
                   for Wl, bt in zip(convW, betas)]).astype(np.float32)
    key = f"nc{int(trace)}"
    if "nc" not in _CACHED:
        _CACHED["nc"] = build_program(debug=_CACHED.get("debug", False))
    nc = _CACHED["nc"]
    in_maps = []
    for c in range(c_.NC):
        xs = np.zeros((c_.NLOC, c_.F_IN), np.float32)
        xs[:c_.NPC] = x[c * c_.NPC:(c + 1) * c_.NPC]
        in_maps.append({
            "x": _bf16(xs), "idx": cores[c]["idx"],
            "Smain": _bf16(cores[c]["Smain"]), "movf": cores[c]["movf"],
            "W0": _bf16(W0), "Wt": _bf16(Wt), "W1": _bf16(W1),
        })
    res = bass_utils.run_bass_kernel_spmd(nc, in_maps, core_ids=list(range(c_.NC)), trace=trace)
    out = np.concatenate([res.results[c]["out"][:c_.NPC] for c in range(c_.NC)], axis=0)
    return out, res


def kernel(x, edge_index, edge_weight, W0, b0, convW, W1, b1):
    """Harness entry: full inputs in, full [N, C] float32 log-softmax out."""
    out, _ = kernel_ex(x, edge_index, edge_weight, W0, b0, convW, W1, b1, trace=False)
    return out



# revision 12
# speedup vs baseline: 1.0950x; 1.0121x over previous
"""GCN2 (GCNII) forward pass on 8 Trainium2 NeuronCores via Bass/Tile.

Strategy (node sharding per sharding hint):
  - dst nodes sharded across 8 cores; per-core slice padded to NLOC rows.
  - full h replica [NC*NLOC, H] bf16 in DRAM on every core, rebuilt each layer
    via AllGather (the "halo exchange" degenerate case: full replication).
  - aggregation: edges grouped by (dst-group of 512, src-core bucket of NLOC
    rows). Per (group, bucket) ONE dma_gather (int16 window indices) pulls
    MAIN_COLS*4+OVF_COLS columns of 128 messages; selection matrices S
    (precomputed on host, streamed from DRAM) turn segment-sum into PE
    matmuls accumulating aggT[f, 512] in PSUM.
  - epilogue folds the initial-residual term (x0a resident in SBUF), applies
    Wt = (1-beta)I + beta*W via one matmul per 128-node block, relu, writes
    the new h slice; AllGather rebuilds the replica.
  - last layer computes logits + log_softmax on-chip, f32 output.
"""
import math
import numpy as np

import concourse.bass as bass
import concourse.bacc as bacc
import concourse.tile as tile
import concourse.mybir as mybir
import concourse.bass_utils as bass_utils
from concourse.masks import make_identity

# ---------------- configuration ----------------
class CFG:
    NC = 8
    N = 100000
    F_IN = 512
    H = 128
    C = 40
    L = 8
    ALPHA = 0.1
    THETA = 0.5
    MAIN_COLS = 2      # 128-edge columns per (block, bucket)
    OVF_COLS = 1       # group-wide overflow columns per (group, bucket)
    GRP = 512
    BLK = 128

    @classmethod
    def derived(cls):
        cls.NPC = cls.N // cls.NC
        cls.NGRP = math.ceil(cls.NPC / cls.GRP)
        cls.NLOC = cls.NGRP * cls.GRP
        cls.NBLK = cls.GRP // cls.BLK
        cls.NCOLS_M = cls.NBLK * cls.MAIN_COLS
        cls.COLS = cls.NCOLS_M + cls.OVF_COLS
        cls.IDX_GJ = cls.COLS * 128
        cls.KT = cls.F_IN // 128


CFG.derived()


def set_sizes(N, F_IN=512, L=8, main_cols=2, ovf_cols=1):
    CFG.N = N
    CFG.F_IN = F_IN
    CFG.L = L
    CFG.MAIN_COLS = main_cols
    CFG.OVF_COLS = ovf_cols
    CFG.derived()


# ---------------- host preprocessing ----------------
def preprocess(edge_index, edge_weight):
    """Per-core device-feed arrays: wrapped int16 gather indices and dense
    selection matrices (bf16) with (1-alpha)-scaled weights baked in."""
    c_ = CFG
    src = edge_index[0].astype(np.int64)
    dst = edge_index[1].astype(np.int64)
    w = edge_weight.astype(np.float32) * (1.0 - c_.ALPHA)
    cores = []
    for c in range(c_.NC):
        m = (dst // c_.NPC) == c
        s_c, d_c, w_c = src[m], dst[m], w[m]
        dl = d_c - c * c_.NPC
        g = dl // c_.GRP
        b = (dl % c_.GRP) // c_.BLK
        j = s_c // c_.NPC
        ilocal = (s_c % c_.NPC).astype(np.int64)
        slot128 = dl % c_.BLK
        slot512 = dl % c_.GRP
        order = np.lexsort((ilocal, b, j, g))
        g, b, j, ilocal, slot128, slot512, w_c = (a[order] for a in (g, b, j, ilocal, slot128, slot512, w_c))
        key = (g * c_.NC + j)
        idxT = np.zeros((c_.NGRP * c_.NC, c_.IDX_GJ), np.int16)
        Smain = np.zeros((c_.NGRP * c_.NC, 128, c_.NCOLS_M * 128), np.float32)
        movf = np.zeros((c_.NGRP * c_.NC, 128, 2 * c_.OVF_COLS), np.float32)
        max_ovf = 0
        # boundaries of (g,j) segments
        seg_start = np.searchsorted(key, np.arange(c_.NGRP * c_.NC), side="left")
        seg_end = np.searchsorted(key, np.arange(c_.NGRP * c_.NC), side="right")
        for gj in range(c_.NGRP * c_.NC):
            s0, s1 = seg_start[gj], seg_end[gj]
            if s0 == s1:
                continue
            bv, iv = b[s0:s1], ilocal[s0:s1]
            s1v, s5v, wv = slot128[s0:s1], slot512[s0:s1], w_c[s0:s1]
            ovf_sel = np.zeros(s1 - s0, bool)
            for bi in range(c_.NBLK):
                bm_idx = np.nonzero(bv == bi)[0]
                cap = c_.MAIN_COLS * 128
                take = bm_idx[:cap]
                base = bi * cap
                nt = len(take)
                idxT[gj, base:base + nt] = iv[take]
                ee = (base + np.arange(nt))
                Smain[gj, ee % 128, (ee // 128) * 128 + s1v[take]] = wv[take]
                if len(bm_idx) > cap:
                    ovf_sel[bm_idx[cap:]] = True
            ov = np.nonzero(ovf_sel)[0]
            ov = ov[np.argsort(iv[ov], kind="stable")]
            no = len(ov)
            max_ovf = max(max_ovf, no)
            if no > c_.OVF_COLS * 128:
                raise OverflowError(f"core {c} gj {gj}: ovf {no} > {c_.OVF_COLS*128}")
            ob = c_.NCOLS_M * 128
            idxT[gj, ob:ob + no] = iv[ov]
            ee = np.arange(no)
            movf[gj, ee % 128, 2 * (ee // 128)] = s5v[ov]
            movf[gj, ee % 128, 2 * (ee // 128) + 1] = wv[ov]
        # wrap idx: flat i -> [i % 16, i // 16], replicated across 8 groups of 16
        wrapped = idxT.reshape(c_.NGRP * c_.NC, c_.IDX_GJ // 16, 16).transpose(0, 2, 1)
        wr = wrapped.transpose(1, 0, 2).reshape(16, -1)
        idx_dev = np.zeros((128, c_.NGRP * c_.NC * (c_.IDX_GJ // 16)), np.int16)
        for q in range(8):
            idx_dev[16 * q:16 * q + 16, :] = wr
        cores.append(dict(idx=idx_dev, Smain=Smain, movf=movf, max_ovf=max_ovf))
    return cores


def _bf16(a):
    import ml_dtypes
    return np.asarray(a, dtype=ml_dtypes.bfloat16)


# ---------------- device program ----------------
def build_program(debug=False):
    c_ = CFG
    nc = bacc.Bacc("TRN2", target_bir_lowering=False, debug=False,
                   enable_asserts=True, num_devices=c_.NC, num_swdge_queues=4)
    bf = mybir.dt.bfloat16
    f32 = mybir.dt.float32
    x_t = nc.dram_tensor("x", [c_.NLOC, c_.F_IN], bf, kind="ExternalInput").ap()
    idx_t = nc.dram_tensor("idx", [128, c_.NGRP * c_.NC * (c_.IDX_GJ // 16)], mybir.dt.int16, kind="ExternalInput").ap()
    sm_t = nc.dram_tensor("Smain", [c_.NGRP * c_.NC, 128, c_.NCOLS_M * 128], bf, kind="ExternalInput").ap()
    mo_t = nc.dram_tensor("movf", [c_.NGRP * c_.NC, 128, 2 * c_.OVF_COLS], f32, kind="ExternalInput").ap()
    w0_t = nc.dram_tensor("W0", [c_.F_IN, c_.H], bf, kind="ExternalInput").ap()
    wt_t = nc.dram_tensor("Wt", [c_.L, c_.H, c_.H], bf, kind="ExternalInput").ap()
    w1_t = nc.dram_tensor("W1", [c_.H, c_.C], bf, kind="ExternalInput").ap()
    out_t = nc.dram_tensor("out", [c_.NLOC, c_.C], f32, kind="ExternalOutput").ap()
    if debug:
        dbg_h0 = nc.dram_tensor("dbg_h0", [c_.NLOC, c_.H], bf, kind="ExternalOutput").ap()
        dbg_rep = nc.dram_tensor("dbg_rep", [c_.NC * c_.NLOC, c_.H], bf, kind="ExternalOutput").ap()
        dbg_outT = nc.dram_tensor("dbg_outT", [128, c_.GRP], f32, kind="ExternalOutput").ap()
        dbg_gt = nc.dram_tensor("dbg_gt", [128, c_.COLS * 128], bf, kind="ExternalOutput").ap()

    slice_b = nc.dram_tensor("slice_b", [c_.NLOC, c_.H], bf, kind="Internal").ap()
    replica = nc.dram_tensor("replica", [c_.NC * c_.NLOC, c_.H], bf, kind="Internal", addr_space="Shared").ap()

    rg = [list(range(c_.NC))]

    with tile.TileContext(nc) as tc:
        with tc.tile_pool(name="res", bufs=1) as res, \
             tc.tile_pool(name="work", bufs=1) as work, \
             tc.tile_pool(name="psum", bufs=1, space="PSUM") as psum:
            # resident tensors
            idx = res.tile([128, c_.NGRP * c_.NC * (c_.IDX_GJ // 16)], mybir.dt.int16)
            nc.sync.dma_start(idx[:], idx_t[:])
            w0 = res.tile([128, c_.KT, c_.H], bf)
            nc.sync.dma_start(w0[:], w0_t[:].rearrange("(k p) f -> p k f", p=128))
            wt = res.tile([128, c_.L, c_.H], bf)
            nc.sync.dma_start(wt[:], wt_t[:].rearrange("l f j -> f l j"))
            w1 = res.tile([128, c_.C], bf)
            nc.sync.dma_start(w1[:], w1_t[:])
            ident = res.tile([128, 128], bf)
            make_identity(nc, ident[:])
            iota = res.tile([128, c_.GRP], f32)
            nc.gpsimd.iota(iota[:], pattern=[[1, c_.GRP]], base=0,
                           channel_multiplier=0,
                           allow_small_or_imprecise_dtypes=True)
            x0a = res.tile([128, c_.NLOC], bf)

            # ---------- layer 0: h0 = relu(x @ W0); x0a = alpha*h0 ----------
            ctx0 = nc.named_scope("layer0"); ctx0.__enter__()
            for g in range(c_.NGRP):
                ph0 = psum.tile([128, c_.GRP], f32, name=f"pga{g%2}", tag="pga", bufs=2, space="PSUM")
                for k in range(c_.KT):
                    xt = work.tile([128, c_.GRP], bf, name=f"xt{k%3}", tag="xt", bufs=3)
                    nc.sync.dma_start(out=xt[:], in_=x_t[g * c_.GRP:(g + 1) * c_.GRP, k * 128:(k + 1) * 128], transpose=True)
                    nc.tensor.matmul(out=ph0[:], lhsT=w0[:, k, :], rhs=xt[:],
                                     start=(k == 0), stop=(k == c_.KT - 1))
                nc.scalar.activation(out=x0a[:, g * c_.GRP:(g + 1) * c_.GRP], in_=ph0[:],
                                     func=mybir.ActivationFunctionType.Relu, scale=c_.ALPHA)
                h0T = work.tile([128, c_.GRP], bf, name=f"h0T{g%2}", tag="h0T", bufs=2)
                nc.scalar.activation(out=h0T[:], in_=ph0[:], func=mybir.ActivationFunctionType.Relu)
                for b in range(c_.NBLK):
                    ptr = psum.tile([128, 128], bf, name=f"ptr{b%2}", tag="p2", bufs=4, space="PSUM")
                    nc.tensor.transpose(out=ptr[:], in_=h0T[:, b * 128:(b + 1) * 128], identity=ident[:])
                    hrow = work.tile([128, 128], bf, name=f"hrow{b%2}", tag="hrow", bufs=4)
                    nc.vector.tensor_copy(out=hrow[:], in_=ptr[:])
                    nc.sync.dma_start(out=slice_b[g * c_.GRP + b * 128: g * c_.GRP + (b + 1) * 128, :], in_=hrow[:])
            ctx0.__exit__(None, None, None)
            ctxag = nc.named_scope("ag0"); ctxag.__enter__()
            nc.gpsimd.collective_compute(
                "AllGather", mybir.AluOpType.bypass, replica_groups=rg,
                ins=[slice_b[:]], outs=[replica[:]])
            ctxag.__exit__(None, None, None)
            if debug:
                dcp = work.tile([128, c_.H], bf, name="dcp", tag="hrow", bufs=4)
                for r in range(c_.NLOC // 128):
                    nc.sync.dma_start(out=dcp[:], in_=slice_b[r * 128:(r + 1) * 128, :])
                    nc.scalar.dma_start(out=dbg_h0[r * 128:(r + 1) * 128, :], in_=dcp[:])
                for r in range(c_.NC * c_.NLOC // 128):
                    nc.sync.dma_start(out=dcp[:], in_=replica[r * 128:(r + 1) * 128, :])
                    nc.scalar.dma_start(out=dbg_rep[r * 128:(r + 1) * 128, :], in_=dcp[:])

            # ---------- conv layers ----------
            for l in range(c_.L):
                last = (l == c_.L - 1)
                ctxl = nc.named_scope(f"conv{l}"); ctxl.__enter__()
                for g in range(c_.NGRP):
                    pga = psum.tile([128, c_.GRP], f32, name=f"pga{g%2}", tag="pga", bufs=2, space="PSUM")
                    pgb = psum.tile([128, c_.GRP], f32, name=f"pgb{g%2}", tag="pgb", bufs=2, space="PSUM")
                    nc.tensor.matmul(out=pga[:], lhsT=ident[:],
                                     rhs=x0a[:, g * c_.GRP:(g + 1) * c_.GRP],
                                     start=True, stop=False, skip_group_check=True)
                    smg = work.tile([128, c_.NC, c_.NCOLS_M * 128], bf, name=f"smg{g%2}", tag="smg", bufs=2)
                    nc.sync.dma_start(out=smg[:], in_=sm_t[g * c_.NC:(g + 1) * c_.NC, :, :].rearrange("j p m -> p j m"))
                    mog = work.tile([128, c_.NC, 2 * c_.OVF_COLS], f32, name=f"mog{g%2}", tag="mog", bufs=2)
                    nc.scalar.dma_start(out=mog[:], in_=mo_t[g * c_.NC:(g + 1) * c_.NC, :, :].rearrange("j p m -> p j m"))
                    for j in range(c_.NC):
                        gj = g * c_.NC + j
                        pagg = pga if j < 4 else pgb
                        gt = work.tile([128, c_.COLS, 128], bf, name=f"gt{j%4}", tag="gt", bufs=10)
                        nc.gpsimd.dma_gather(
                            out_ap=gt[:], in_ap=replica[j * c_.NLOC:(j + 1) * c_.NLOC, :],
                            idxs_ap=idx[:, gj * (c_.IDX_GJ // 16):(gj + 1) * (c_.IDX_GJ // 16)],
                            num_idxs=c_.IDX_GJ, num_idxs_reg=c_.IDX_GJ, elem_size=c_.H,
                            queue_num=j % 4, single_packet=False)
                        sm = smg[:, j, :]
                        mo = mog[:, j, :]
                        so = work.tile([128, c_.OVF_COLS * c_.GRP], bf, name=f"so{j%3}", tag="so", bufs=4)
                        for oc in range(c_.OVF_COLS):
                            nc.vector.tensor_scalar(
                                out=so[:, oc * c_.GRP:(oc + 1) * c_.GRP], in0=iota[:],
                                scalar1=mo[:, 2 * oc:2 * oc + 1],
                                scalar2=mo[:, 2 * oc + 1:2 * oc + 2],
                                op0=mybir.AluOpType.is_equal, op1=mybir.AluOpType.mult)
                        # j==4: overflow column first; its start=True (512-wide,
                        # covers the whole bank) resets bank B - no init matmul.
                        if j == 4:
                            for oc in range(c_.OVF_COLS):
                                nc.tensor.matmul(out=pagg[:], lhsT=gt[:, c_.NCOLS_M + oc, :],
                                                 rhs=so[:, oc * c_.GRP:(oc + 1) * c_.GRP],
                                                 start=(oc == 0), stop=False,
                                                 skip_group_check=True)
                        for cc in range(c_.NCOLS_M):
                            bi = cc // c_.MAIN_COLS
                            nc.tensor.matmul(out=pagg[:, bi * 128:(bi + 1) * 128],
                                             lhsT=gt[:, cc, :], rhs=sm[:, cc * 128:(cc + 1) * 128],
                                             start=False, stop=False, skip_group_check=True)
                        if j != 4:
                            for oc in range(c_.OVF_COLS):
                                nc.tensor.matmul(out=pagg[:], lhsT=gt[:, c_.NCOLS_M + oc, :],
                                                 rhs=so[:, oc * c_.GRP:(oc + 1) * c_.GRP],
                                                 start=False,
                                                 stop=((j == 3 or j == c_.NC - 1) and oc == c_.OVF_COLS - 1),
                                                 skip_group_check=True)
                    aggb = work.tile([128, c_.GRP], bf, name=f"aggb{g%2}", tag="aggb", bufs=3)
                    nc.scalar.activation(out=aggb[:], in_=pgb[:], func=mybir.ActivationFunctionType.Copy)
                    outT = work.tile([128, c_.GRP], bf, name=f"outT{g%2}", tag="outT", bufs=3)
                    nc.vector.tensor_tensor(out=outT[:], in0=pga[:], in1=aggb[:],
                                            op=mybir.AluOpType.add)
                    hng = None
                    for b in range(c_.NBLK):
                        if not last:
                            p2 = psum.tile([128, 128], f32, name=f"p2{b%3}", tag="p2", bufs=4, space="PSUM")
                            nc.tensor.matmul(out=p2[:], lhsT=outT[:, b * 128:(b + 1) * 128],
                                             rhs=wt[:, l, :], start=True, stop=True)
                            if hng is None:
                                hng = work.tile([128, c_.NBLK, 128], bf, name=f"hng{g%2}", tag="hng", bufs=2)
                            nc.scalar.activation(out=hng[:, b, :], in_=p2[:], func=mybir.ActivationFunctionType.Relu)
                            if b == c_.NBLK - 1:
                                nc.sync.dma_start(
                                    out=slice_b[g * c_.GRP:(g + 1) * c_.GRP, :].rearrange("(b p) h -> p b h", p=128),
                                    in_=hng[:])
                        else:
                            p2 = psum.tile([128, 128], f32, name=f"p2{b%3}", tag="p2", bufs=4, space="PSUM")
                            nc.tensor.matmul(out=p2[:], lhsT=wt[:, l, :],
                                             rhs=outT[:, b * 128:(b + 1) * 128], start=True, stop=True)
                            h8T = work.tile([128, 128], bf, name=f"h8T{b%2}", tag="hrow", bufs=4)
                            nc.scalar.activation(out=h8T[:], in_=p2[:], func=mybir.ActivationFunctionType.Relu)
                            plg = psum.tile([128, 128], f32, name=f"plg{b%3}", tag="p2", bufs=4, space="PSUM")
                            nc.tensor.matmul(out=plg[:, :c_.C], lhsT=h8T[:], rhs=w1[:], start=True, stop=True)
                            negm = work.tile([128, 1], f32, name=f"negm{b%2}", tag="negm", bufs=4)
                            nc.vector.reduce_max(out=negm[:], in_=plg[:, :c_.C], axis=mybir.AxisListType.X, negate=True)
                            esc = work.tile([128, c_.C], bf, name=f"esc{b%2}", tag="esc", bufs=2)
                            ssum = work.tile([128, 1], f32, name=f"ssum{b%2}", tag="ssum", bufs=4)
                            nc.scalar.activation(out=esc[:], in_=plg[:, :c_.C], func=mybir.ActivationFunctionType.Exp,
                                                 bias=negm[:, :1], accum_out=ssum[:, :1])
                            lsum = work.tile([128, 1], f32, name=f"lsum{b%2}", tag="lsum", bufs=4)
                            nc.scalar.activation(out=lsum[:], in_=ssum[:], func=mybir.ActivationFunctionType.Ln)
                            fin = work.tile([128, c_.C], f32, name=f"fin{b%2}", tag="fin", bufs=4)
                            nc.vector.tensor_scalar(out=fin[:], in0=plg[:, :c_.C],
                                                    scalar1=negm[:, :1], scalar2=lsum[:, :1],
                                                    op0=mybir.AluOpType.add, op1=mybir.AluOpType.subtract)
                            eng = nc.sync if b % 2 == 0 else nc.scalar
                            eng.dma_start(out=out_t[g * c_.GRP + b * 128: g * c_.GRP + (b + 1) * 128, :], in_=fin[:])
                ctxl.__exit__(None, None, None)
                if not last:
                    ctxa = nc.named_scope(f"ag{l+1}"); ctxa.__enter__()
                    nc.gpsimd.collective_compute(
                        "AllGather", mybir.AluOpType.bypass, replica_groups=rg,
                        ins=[slice_b[:]], outs=[replica[:]])
                    ctxa.__exit__(None, None, None)
    nc.compile()
    return nc


# ---------------- end-to-end host entry ----------------
_CACHED = {}

def kernel_ex(x, edge_index, edge_weight, W0, b0, convW, W1, b1, trace=False):
    c_ = CFG
    x = np.asarray(x); edge_index = np.asarray(edge_index); edge_weight = np.asarray(edge_weight)
    W0 = np.asarray(W0); convW = np.asarray(convW); W1 = np.asarray(W1)
    b0 = np.asarray(b0); b1 = np.asarray(b1)
    assert np.abs(b0).max() == 0.0 and np.abs(b1).max() == 0.0, "nonzero biases unsupported"
    while True:
        try:
            cores = preprocess(edge_index, edge_weight)
            break
        except OverflowError:
            set_sizes(CFG.N, F_IN=CFG.F_IN, L=CFG.L, main_cols=CFG.MAIN_COLS,
                      ovf_cols=CFG.OVF_COLS + 1)
            _CACHED.pop("nc", None)
    betas = np.log(c_.THETA / np.arange(1, c_.L + 1, dtype=np.float64) + 1.0)
    Wt = np.stack([(1 - bt) * np.eye(c_.H) + bt * Wl.astype(np.float64)
                   for Wl, bt in zip(convW, betas)]).astype(np.float32)
    key = f"nc{int(trace)}"
    if "nc" not in _CACHED:
        _CACHED["nc"] = build_program(debug=_CACHED.get("debug", False))
    nc = _CACHED["nc"]
    in_maps = []
    for c in range(c_.NC):
        xs = np.zeros((c_.NLOC, c_.F_IN), np.float32)
        xs[:c_.NPC] = x[c * c_.NPC:(c + 1) * c_.NPC]
        in_maps.append({
            "x": _bf16(xs), "idx": cores[c]["idx"],
            "Smain": _bf16(cores[c]["Smain"]), "movf": cores[c]["movf"],
            "W0": _bf16(W0), "Wt": _bf16(Wt), "W1": _bf16(W1),
        })
    res = bass_utils.run_bass_kernel_spmd(nc, in_maps, core_ids=list(range(c_.NC)), trace=trace)
    out = np.concatenate([res.results[c]["out"][:c_.NPC] for c in range(c_.NC)], axis=0)
    return out, res


def kernel(x, edge_index, edge_weight, W0, b0, convW, W1, b1):
    """Harness entry: full inputs in, full [N, C] float32 log-softmax out."""
    out, _ = kernel_ex(x, edge_index, edge_weight, W0, b0, convW, W1, b1, trace=False)
    return out

